# revision 1
# baseline (speedup 1.0000x reference)
"""LocallyConnected1d Trainium2 kernel (v4: x-stationary fp32r matmuls,
fused kernel taps, host-pretiled weights).

out[b, o, l] = sum_{c,k} x[b, c, l+k] * weight[o, c, l, k] + bias[o, l]
  x: (32, 128, 2050) f32, weight: (128, 128, 2048, 3) f32, bias: (128, 2048) f32
  out: (32, 128, 2048) f32

Sharding: sequence-parallel over L across 8 cores (each core owns 256 output
positions, its private 50.3 MB weight slice, a 258-wide x window, and a
transposed bias slice).  Weight streaming from HBM is the roofline; the host
lays each core's weight shard out as the exact per-window SBUF tile images
(c, o, l, k) so every weight DMA descriptor is one contiguous 24 KB run.

Per-core compute: out.T[b, l, o] = sum_c x[b, c, m] * W[o, c, l, m-l] per x
column m.  The x column is the PE stationary operand (K=128 c, M=32 b); the
weights are the moving operand.  For one column m the contributions to
l = m-2..m form an anti-diagonal of the weight tile's (l, k) plane — stride 2
in the flattened l*3+k axis — so all three taps fuse into ONE matmul with
N = 3*128 = 384 moving columns.  float32r (single-pass fp32) streams
1 row/cycle at N >= 256.

PSUM: one bank holds out.T slice (32 b, 4 l, 128 o).  Each bank takes 7
matmuls: a K=1 ones x biasT matmul (start=True clears the bank, seeds the
bias, sets every has_written bit), then 6 weight matmuls (m = 4j..4j+5
clipped to the bank; per-element has_written makes them pure accumulates).
DVE copies each bank to (b, l, o)-ordered staging; the out DMA writes
contiguous runs and the host transposes after gather.
"""

import numpy as np

import concourse.bass as bass
import concourse.mybir as mybir
import concourse.tile as tile
from concourse.vector_clock import ScopedClock, VectorClock
from concourse.bass_utils import run_bass_kernel_spmd

# ---------------------------------------------------------------------------
# Environment patches
# ---------------------------------------------------------------------------

# The walrus build in this image rejects instructions with >1 sem wait; the
# Tile tail drain carries one wait per logical processor.  Split them into
# single-wait nops on SP before the drain.
def _patched_drain_and_barrier(self, tick_clock, wait_clock):
    gc = tick_clock.global_clock
    n = len(gc)
    for proc in range(n):
        t = gc[proc]
        if t <= 0:
            continue
        single = VectorClock([0] * n)
        single.require_at_least(proc, t)
        inst = self.nc.sync.nop(hint="tail_drain_wait")
        wait_clock.add_sem_waits(inst.ins, ScopedClock({None: single}))
    self.nc.sync.drain()
    self.nc.all_engine_barrier()
    assert self.sems is not None
    popped = self.nc._tile_sem_poison_stack.pop()
    assert popped is self._sem_poison
    # Clear sems WITHOUT the trailing all-engine barrier: the clear runs on
    # one engine after the barrier above, and nothing after it reads sems.
    self.nc.clear_and_free_semaphores(list(self.sems.allocated().values()))


if not getattr(tile.TileContext, "_drain_patch_applied", False):
    tile.TileContext._drain_and_barrier = _patched_drain_and_barrier
    tile.TileContext._drain_patch_applied = True


def _split_multi_waits(nc: bass.Bass) -> int:
    """Hoist all but the last wait of any multi-wait instruction onto
    single-wait nops inserted just before it in its engine's program order
    (the hardware takes one sem wait per instruction; this walrus build
    rejects multi-wait instructions instead of splitting them)."""
    n_split = 0
    for f in nc.m.functions:
        for bb in f.blocks:
            insts = list(bb.instructions)
            out = []
            for inst in insts:
                si = inst.sync_info
                if si is not None and len(si.on_wait) > 1:
                    waits = list(si.on_wait)
                    for w in waits[:-1]:
                        nop = mybir.InstNoOp(
                            name=nc.get_next_instruction_name(),
                            engine=inst.engine,
                            ins=[],
                            outs=[],
                            sync_info=mybir.SyncInfo(on_wait=[w], on_update=[]),
                        )
                        out.append(nop)
                    si.on_wait = [waits[-1]]
                    n_split += 1
                out.append(inst)
            bb.instructions = out
    return n_split

# ---------------------------------------------------------------------------
# Problem constants (hardcoded from the module spec)
# ---------------------------------------------------------------------------
N_CORES = 8
B = 32
CIN = 128
COUT = 128
L = 2048
KS = 3
W_FULL = 2050

LSH = L // N_CORES          # 256 output positions per core
WW = LSH + KS - 1           # 258-wide x window per core

LT = 16                     # l positions per weight tile / staging window
NWIN = LSH // LT            # 16 windows per core
BANKL = 4                   # l positions per PSUM bank (4*128 = 512 fp32)
NBANK = LT // BANKL         # 4 banks per window
WFREE = COUT * LT * KS      # weight tile free size (6144 fp32 = 24 KB)

F32 = mybir.dt.float32
F32R = mybir.dt.float32r


def _build_nc(split: bool = True) -> bass.Bass:
    nc = bass.Bass()

    x_d = nc.declare_dram_parameter("x", [B, CIN, WW], F32R, isOutput=False)
    wt_d = nc.declare_dram_parameter("wt", [NWIN, CIN, WFREE], F32R,
                                     isOutput=False)
    bt_d = nc.declare_dram_parameter("biasT", [LSH, COUT], F32R, isOutput=False)
    ones_d = nc.declare_dram_parameter("ones", [1, B], F32R, isOutput=False)
    # (b, l, o) layout: staging DMAs out as contiguous runs; the host
    # transposes back after gather.
    out_d = nc.declare_dram_parameter("out", [B, LSH, COUT], F32, isOutput=True)

    with tile.TileContext(nc) as tc:
        with (
            tc.tile_pool(name="xp", bufs=1) as xp,
            tc.tile_pool(name="cp", bufs=1) as cp,
            tc.tile_pool(name="wp", bufs=4) as wp,
            tc.tile_pool(name="bp", bufs=3) as bp,
            tc.tile_pool(name="sp", bufs=3) as sp,
            tc.tile_pool(name="pp", bufs=8, space="PSUM") as pp,
        ):
            # Persistent x in (c, b, w) layout: the stationary operand for
            # column m is x_sb[:, :, m] (K=128 c, M=32 b).  DMA runs 1032 B.
            x_sb = xp.tile([CIN, B, WW], F32R)
            # split so window 0's matmuls only wait on the first columns
            nc.sync.dma_start(x_sb[:, :, 0:2 * LT + 2],
                              x_d[:, :, 0:2 * LT + 2]
                              .rearrange("b c w -> c b w"))
            nc.sync.dma_start(x_sb[:, :, 2 * LT + 2:WW],
                              x_d[:, :, 2 * LT + 2:WW]
                              .rearrange("b c w -> c b w"))

            ones = cp.tile([1, B], F32R)
            nc.sync.dma_start(ones[:], ones_d[:])

            for lc in range(NWIN):
                # weight tile (c, o, l*k); one contiguous 24 KB run/partition
                w_t = wp.tile([CIN, COUT, LT * KS], F32R, tag="w", name="w_t")
                nc.sync.dma_start(
                    w_t[:].rearrange("c o f -> c (o f)"), wt_d[lc]
                )

                # biasT rows for this window, flattened on partition 0
                btile = bp.tile([1, LT * COUT], F32R, tag="bt",
                                name=f"bt_{lc}")
                nc.sync.dma_start(
                    btile[:],
                    bt_d[lc * LT:(lc + 1) * LT, :]
                    .rearrange("l o -> (l o)")[None, :],
                )

                st = sp.tile([B, LT, COUT], F32, tag="st", name=f"st_{lc}")

                for jb in range(NBANK):
                    ps = pp.tile([B, BANKL, COUT], F32, tag="ps", name="ps")
                    lw0 = jb * BANKL              # window-local l of bank start

                    # bias init: out[b, (l, o)] = 1[b] * biasT[(l, o)];
                    # start=True clears the bank and sets has_written.
                    boff = lw0 * COUT
                    nc.tensor.matmul(
                        ps[:].rearrange("b l o -> b (l o)"),
                        ones[:],
                        btile[0:1, boff:boff + BANKL * COUT],
                        start=True,
                        stop=False,
                        skip_group_check=True,
                    )

                    # six weight matmuls: x columns m = bank start .. +5
                    for d in range(BANKL + KS - 1):
                        mw = lw0 + d                  # window-local x column
                        m = lc * LT + mw              # shard-local x column
                        lo = max(lw0, mw - (KS - 1))  # window-local l' range
                        hi = min(lw0 + BANKL - 1, mw)
                        nl = hi - lo + 1
                        # anti-diagonal AP over the weight tile: element
                        # (o, l', k=mw-l') at o*(LT*KS) + l'*3 + (mw-l')
                        # -> l' step 2, o step LT*KS
                        rhs = bass.AP(
                            w_t[:].tensor,
                            lo * KS + (mw - lo),
                            [[COUT * LT * KS, CIN], [2, nl], [LT * KS, COUT]],
                        )
                        nc.tensor.matmul(
                            ps[:, lo - lw0:hi - lw0 + 1, :],
                            x_sb[:, :, m],
                            rhs,
                            start=False,
                            stop=(d == BANKL + KS - 2),
                            skip_group_check=True,
                        )

                    # PSUM (b, l, o) -> staging (b, l, o), plain copy
                    nc.vector.tensor_copy(
                        st[:, lw0:lw0 + BANKL, :],
                        ps[:],
                    )

                if lc < NWIN - 1:
                    nc.scalar.dma_start(out_d[:, lc * LT:(lc + 1) * LT, :],
                                        st[:])
                else:
                    # last window: per-bank flushes so the kernel tail is one
                    # 64 KB transfer instead of a whole-window 256 KB one
                    for jb in range(NBANK):
                        l0 = lc * LT + jb * BANKL
                        nc.scalar.dma_start(
                            out_d[:, l0:l0 + BANKL, :],
                            st[:, jb * BANKL:(jb + 1) * BANKL, :])

    if split:
        _split_multi_waits(nc)
    return nc


_NC_CACHE = None


def _get_nc() -> bass.Bass:
    global _NC_CACHE
    if _NC_CACHE is None:
        _NC_CACHE = _build_nc()
    return _NC_CACHE


def _tile_weights(w_shard: np.ndarray) -> np.ndarray:
    """(COUT, CIN, LSH, KS) -> (NWIN, CIN, COUT*LT*KS) per-window SBUF tile
    images: wt[n, c, o*LT*KS + l*KS + k] = w_shard[o, c, n*LT + l, k]."""
    w = w_shard.transpose(1, 0, 2, 3)                  # (CIN, COUT, LSH, KS)
    w = w.reshape(CIN, COUT, NWIN, LT, KS)
    w = w.transpose(2, 0, 1, 3, 4)                     # (NWIN, CIN, COUT, LT, KS)
    return np.ascontiguousarray(w.reshape(NWIN, CIN, WFREE))


def shard_inputs(x, weight, bias):
    x = np.asarray(x, dtype=np.float32)
    weight = np.asarray(weight, dtype=np.float32)
    bias = np.asarray(bias, dtype=np.float32)
    in_maps = []
    for i in range(N_CORES):
        l0 = i * LSH
        in_maps.append({
            "x": np.ascontiguousarray(x[:, :, l0:l0 + WW]),
            "wt": _tile_weights(weight[:, :, l0:l0 + LSH, :]),
            "biasT": np.ascontiguousarray(bias[:, l0:l0 + LSH].T),
            "ones": np.ones((1, B), dtype=np.float32),
        })
    return in_maps


def gather_output(results):
    out = np.empty((B, COUT, L), dtype=np.float32)
    for i in range(N_CORES):
        out[:, :, i * LSH:(i + 1) * LSH] = results[i]["out"].transpose(0, 2, 1)
    return out


def kernel(x, weight, bias):
    nc = _get_nc()
    in_maps = shard_inputs(x, weight, bias)
    res = run_bass_kernel_spmd(nc, in_maps, core_ids=list(range(N_CORES)),
                               trace=False)
    return gather_output(res.results)

